# revision 17
# baseline (speedup 1.0000x reference)
"""Trainium2 Bass kernel for sparse (top-k) attention with relative-position
bias and gating, sharded over 8 NeuronCores by (batch x head).

Layout per core c: heads [2c, 2c+1] for all 4 batches. Each core computes a
partial output contribution out_c = concat(head_outs) @ Wo[head_rows]; the
host sums the 8 partials and adds bo.

Pipeline per (b, h), per 128-query tile:
  scores   = (q*SCALE) @ k^T + gather(P, toeplitz)   [PE + DMA-diagonal]
  top-64   threshold t' via per-chunk max8 candidates + 8 max8/match_replace
           rounds on the 256 candidates                [DVE]
  exp/mask e = exp(s - t') masked below t', row-sum    [DVE mask + ACT exp]
  attn     a = e * (1/den) * gating                    [DVE, bf16]
  out_h    = (a @ v) via PE transpose + V^T A^T matmul [PE]
"""

import numpy as np

import concourse.bass as bass
import concourse.mybir as mybir
from concourse.bass_types import AP
from concourse.tile import TileContext
from concourse.bass_utils import run_bass_kernel_spmd
from concourse.vector_clock import ScopedClock

F32 = mybir.dt.float32
BF16 = mybir.dt.bfloat16
Alu = mybir.AluOpType
Act = mybir.ActivationFunctionType

B, N, DIM, H, DH = 4, 1024, 1024, 16, 64
INNER = H * DH
MAX_POS = 256
TOPK = 64
SCALE = DH ** -0.5
HPC = 2            # heads per core
NCORES = 8
QT = 128           # queries per tile
NQT = N // QT      # 8 query tiles
NEG = -1.0e30
PW = 512           # padded P_ext row width (clamp128 | rev256 | clamp128)


# ---------------------------------------------------------------------------
# workarounds: this walrus build rejects instructions with >1 sem wait
# ---------------------------------------------------------------------------

def _patched_drain_and_barrier(self, tick_clock, wait_clock):
    nc = self.nc
    probe = nc.sync.nop()
    wait_clock.add_sem_waits(probe.ins, ScopedClock({None: tick_clock.global_clock}))
    waits = list(probe.ins.sync_info.on_wait)
    if len(waits) > 1:
        si = probe.ins.sync_info
        si.on_wait = [waits[0]]
        probe.ins.sync_info = si
        sem_by_name = {s.name: s for s in self.sems.allocated().values()}
        for w in waits[1:]:
            h = sem_by_name.get(w.ant_name)
            if h is None:
                for s in self.sems.allocated().values():
                    if getattr(s, "sem_id", None) == w.id:
                        h = s
                        break
            assert h is not None, f"no handle for {w}"
            nc.sync.wait_ge(h, w.wait_value)
    nc.sync.drain()
    nc.all_engine_barrier()
    assert self.sems is not None
    popped = nc._tile_sem_poison_stack.pop()
    assert popped is self._sem_poison
    nc.clear_and_free_semaphores(list(self.sems.allocated().values()))
    nc.all_engine_barrier()


def _apply_tile_patch():
    import concourse.tile as tile_mod

    tile_mod.TileContext._drain_and_barrier = _patched_drain_and_barrier


def split_excess_waits(nc, max_waits: int = 1):
    eng_by_type = {
        mybir.EngineType.PE: nc.tensor,
        mybir.EngineType.DVE: nc.vector,
        mybir.EngineType.Activation: nc.scalar,
        mybir.EngineType.Pool: nc.gpsimd,
        mybir.EngineType.SP: nc.sync,
    }
    for _, bbb in list(nc.bb_map.items()):
        bb = bbb.bb if hasattr(bbb, "bb") else bbb
        insts = bb.instructions
        i = 0
        while i < len(insts):
            inst = insts[i]
            si = getattr(inst, "sync_info", None)
            if si is not None and si.on_wait and len(si.on_wait) > max_waits:
                waits = list(si.on_wait)
                si.on_wait = waits[:max_waits]
                inst.sync_info = si
                excess = waits[max_waits:]
                eng = eng_by_type[inst.engine]
                nops = []
                for j in range(0, len(excess), max_waits):
                    nop_bi = eng.nop()
                    nop_inst = nop_bi.ins if hasattr(nop_bi, "ins") else nop_bi
                    cur = nc.cur_bb.bb.instructions
                    assert cur[-1] is nop_inst
                    cur.pop()
                    nsi = nop_inst.sync_info
                    if nsi is None:
                        nsi = mybir.SyncInfo(on_wait=[], on_update=[])
                    nsi.on_wait = excess[j:j + max_waits]
                    nop_inst.sync_info = nsi
                    nops.append(nop_inst)
                for k, nop_inst in enumerate(nops):
                    insts.insert(i + k, nop_inst)
                i += len(nops)
            i += 1


# ---------------------------------------------------------------------------
# program builder (SPMD: identical program on all 8 cores)
# ---------------------------------------------------------------------------

def build_program():
    nc = bass.Bass("TRN2")

    xT = nc.dram_tensor("xT", [B, DIM, N], F32, kind="ExternalInput")
    wq = nc.dram_tensor("wq", [DIM, HPC * DH], F32, kind="ExternalInput")
    wk = nc.dram_tensor("wk", [DIM, HPC * DH], F32, kind="ExternalInput")
    wv = nc.dram_tensor("wv", [DIM, HPC * DH], F32, kind="ExternalInput")
    bqk = nc.dram_tensor("bqk", [HPC * DH, 2], F32, kind="ExternalInput")
    bvb = nc.dram_tensor("bvb", [1, HPC * DH], F32, kind="ExternalInput")
    wo = nc.dram_tensor("wo", [HPC * DH, DIM], BF16, kind="ExternalInput")
    reT = nc.dram_tensor("reT", [DH, MAX_POS], F32, kind="ExternalInput")
    gat = nc.dram_tensor("gat", [B, HPC, N, N], BF16, kind="ExternalInput")
    ident_in = nc.dram_tensor("ident", [128, 128], BF16, kind="ExternalInput")
    identf_in = nc.dram_tensor("identf", [128, 128], F32, kind="ExternalInput")
    out = nc.dram_tensor("out", [B, N, DIM], F32, kind="ExternalOutput")
    # Toeplitz-padded rel-pos bias rows, 512 wide:
    #   cols [0,128)   = P[i,255] broadcast (clamp for j <= i)
    #   cols [128,384) = P_rev[i, :] = P[i, 255..0]
    #   cols [384,512) = P[i,0]   broadcast (clamp for j > i+256)
    # The diagonal read with row pitch PW-1 slides one col per query row, so
    # element (r, jj) of a band window lands on col 127 + (j - i).
    pexts = [nc.dram_tensor(f"pext{i}", [N, PW], BF16, kind="Internal")
             for i in range(4)]

    from contextlib import ExitStack
    with TileContext(nc) as tc, ExitStack() as es:
        cpool = es.enter_context(tc.tile_pool(name="consts", bufs=1))
        wq_s = cpool.tile([128, 8, HPC * DH], F32, tag="wq")
        wk_s = cpool.tile([128, 8, HPC * DH], F32, tag="wk")
        wv_s = cpool.tile([128, 8, HPC * DH], F32, tag="wv")
        nc.sync.dma_start(out=wq_s[:], in_=wq.rearrange("(c p) n -> p c n", p=128))
        nc.sync.dma_start(out=wk_s[:], in_=wk.rearrange("(c p) n -> p c n", p=128))
        nc.sync.dma_start(out=wv_s[:], in_=wv.rearrange("(c p) n -> p c n", p=128))
        wo_s = cpool.tile([128, DIM], BF16, tag="wo")
        nc.sync.dma_start(out=wo_s[:], in_=wo[:, :])
        reT_s = cpool.tile([128, MAX_POS], F32, tag="reT")
        nc.sync.dma_start(out=reT_s[0:DH, :], in_=reT[:, :])
        nc.sync.dma_start(out=reT_s[DH:128, :], in_=reT[:, :])
        bqk_s = cpool.tile([128, 2], F32, tag="bqk")
        nc.sync.dma_start(out=bqk_s[:], in_=bqk[:, :])
        bv_s = cpool.tile([128, HPC * DH], F32, tag="bv")
        nc.sync.dma_start(
            out=bv_s[:],
            in_=AP(tensor=bvb, offset=0, ap=[[0, 128], [1, HPC * DH]]),
        )
        ident = cpool.tile([128, 128], BF16, tag="ident")
        nc.sync.dma_start(out=ident[:], in_=ident_in[:, :])
        identf = cpool.tile([128, 128], F32, tag="identf")
        nc.sync.dma_start(out=identf[:], in_=identf_in[:, :])

        xt_pool = es.enter_context(tc.tile_pool(name="xt", bufs=2))
        qkv_pool = es.enter_context(tc.tile_pool(name="qkv", bufs=2))
        ppool = es.enter_context(tc.tile_pool(name="pp", bufs=3))
        clpool = es.enter_context(tc.tile_pool(name="clamp", bufs=18))
        spool = es.enter_context(tc.tile_pool(name="scores", bufs=3))
        mpool = es.enter_context(tc.tile_pool(name="mneg", bufs=2))
        epool = es.enter_context(tc.tile_pool(name="ea", bufs=3))
        gpool = es.enter_context(tc.tile_pool(name="gate", bufs=3))
        small = es.enter_context(tc.tile_pool(name="small", bufs=4))
        atp = es.enter_context(tc.tile_pool(name="atp", bufs=2))
        repp = es.enter_context(tc.tile_pool(name="repp", bufs=3))
        otp = es.enter_context(tc.tile_pool(name="otp", bufs=2))
        outp = es.enter_context(tc.tile_pool(name="outp", bufs=2))

        ps_mm = es.enter_context(tc.tile_pool(name="ps_mm", bufs=2, space="PSUM"))
        ps_s = es.enter_context(tc.tile_pool(name="ps_s", bufs=2, space="PSUM"))
        ps_t = es.enter_context(tc.tile_pool(name="ps_t", bufs=2, space="PSUM"))
        ps_av = es.enter_context(tc.tile_pool(name="ps_av", bufs=2, space="PSUM"))

        def emit_xt(b):
            xt = xt_pool.tile([128, 8, N], F32, tag="xt")
            for mc in range(8):
                nc.sync.dma_start(out=xt[:, mc, :], in_=xT[b, mc * 128:(mc + 1) * 128, :])
            return xt

        def proj_units(xt):
            """qT/kT/V projection for one batch as a list of emission units."""
            qT = qkv_pool.tile([128, N], F32, tag="qT")
            kT = qkv_pool.tile([128, N], F32, tag="kT")
            V = qkv_pool.tile([128, 8, HPC * DH], BF16, tag="V")
            units = []
            for dst, w_s, col in ((qT, wq_s, 0), (kT, wk_s, 1)):
                for half in range(2):
                    def u(dst=dst, w_s=w_s, col=col, half=half):
                        ps = ps_mm.tile([128, 512], F32, tag="mm512")
                        for mc in range(8):
                            nc.tensor.matmul(
                                ps[:],
                                lhsT=w_s[:, mc, :],
                                rhs=xt[:, mc, half * 512:(half + 1) * 512],
                                start=(mc == 0),
                                stop=(mc == 7),
                            )
                        nc.scalar.activation(
                            dst[:, half * 512:(half + 1) * 512], ps[:],
                            Act.Identity, bias=bqk_s[:, col:col + 1])
                    units.append(u)
            for jt in range(8):
                def u(jt=jt):
                    ps = ps_mm.tile([128, 512], F32, tag="mm512")
                    for mc in range(8):
                        nc.tensor.matmul(
                            ps[:, 0:HPC * DH],
                            lhsT=xt[:, mc, jt * 128:(jt + 1) * 128],
                            rhs=wv_s[:, mc, :],
                            start=(mc == 0),
                            stop=(mc == 7),
                        )
                    nc.vector.tensor_tensor(out=V[:, jt, :], in0=ps[:, 0:HPC * DH], in1=bv_s[:], op=Alu.add)
                units.append(u)
            return qT, kT, V, units

        def p_units(qT, hs, pext):
            """P = q_scaled @ rel^T and the padded reversed rows, per tile."""
            clamps = []
            units = []
            for qi in range(NQT):
                cl = clpool.tile([128, 2], F32, tag="clamp")
                clamps.append(cl)

                def u(qi=qi, cl=cl):
                    ps = ps_mm.tile([128, 512], F32, tag="mm512")
                    nc.tensor.matmul(
                        ps[:, 0:MAX_POS],
                        lhsT=qT[hs:hs + DH, qi * 128:(qi + 1) * 128],
                        rhs=reT_s[hs:hs + DH, :],
                        start=True, stop=True,
                    )
                    # clamp2: col0 = P[i,0] (right clamp), col1 = P[i,255] (left)
                    nc.scalar.activation(
                        cl[:],
                        AP(tensor=ps.tensor, offset=ps.offset,
                           ap=[list(ps.ap[0]), [255, 2]]),
                        Act.Copy)
                    pb = ppool.tile([128, PW], BF16, tag="pb")
                    nc.scalar.activation(
                        pb[:, 0:128],
                        AP(tensor=ps.tensor, offset=ps.offset + 255,
                           ap=[list(ps.ap[0]), [0, 128]]),
                        Act.Copy)
                    nc.scalar.activation(pb[:, 128:128 + MAX_POS],
                                         ps[:, 0:MAX_POS][:, ::-1], Act.Copy)
                    nc.scalar.activation(
                        pb[:, 128 + MAX_POS:PW],
                        AP(tensor=ps.tensor, offset=ps.offset,
                           ap=[list(ps.ap[0]), [0, PW - 128 - MAX_POS]]),
                        Act.Copy)
                    nc.sync.dma_start(out=pext[qi * 128:(qi + 1) * 128, :], in_=pb[:])
                units.append(u)
            return clamps, units

        def outproj_units(b, OT):
            units = []
            for qi in range(NQT):
                def u(qi=qi):
                    i0 = qi * 128
                    ob = outp.tile([128, DIM], F32, tag="ob")
                    for half in range(2):
                        o_ps = ps_mm.tile([128, 512], F32, tag="mm512")
                        nc.tensor.matmul(
                            o_ps[:],
                            lhsT=OT[:, i0:i0 + 128],
                            rhs=wo_s[:, half * 512:(half + 1) * 512],
                            start=True, stop=True,
                        )
                        nc.scalar.activation(ob[:, half * 512:(half + 1) * 512],
                                             o_ps[:], Act.Copy)
                    nc.sync.dma_start(out=out[b, i0:i0 + 128, :], in_=ob[:])
                units.append(u)
            return units

        def tile_loop(b, h, qT, kT, V, OT, clamps, pext, extra_units):
            """Software-pipelined tile loop; extra_units are interleaved one
            slot at a time so PE's in-order stream never serializes whole
            projection/P phases at section boundaries."""
            hs = h * DH
            S_l, negt_l, avrep_l, A_l = {}, {}, {}, {}

            def stage_scores(qi):
                i0 = qi * 128
                w = min(384, N - i0)  # band: blocks {qi, qi+1, qi+2}
                bias_t = ppool.tile([128, 384], BF16, tag="bias")
                diag = AP(
                    tensor=pext,
                    offset=i0 * PW + 127,
                    ap=[[PW - 1, 128], [1, w]],
                )
                nc.sync.dma_start(out=bias_t[:, 0:w], in_=diag)

                S = spool.tile([128, N], F32, tag="S")
                regions = [(0, i0, 1), (i0, i0 + w, None), (i0 + w, N, 0)]
                for half in range(2):
                    h0c = half * 512
                    s_ps = ps_s.tile([128, 512], F32, tag="s")
                    for (lo, hi, bc) in regions:
                        a, bnd = max(lo, h0c), min(hi, h0c + 512)
                        if a >= bnd:
                            continue
                        nc.tensor.matmul(
                            s_ps[:, a - h0c:bnd - h0c],
                            lhsT=qT[hs:hs + DH, i0:i0 + 128],
                            rhs=kT[hs:hs + DH, a:bnd],
                            start=True, stop=(bc is not None),
                        )
                        if bc is None:
                            nc.tensor.matmul(
                                s_ps[:, a - h0c:bnd - h0c],
                                lhsT=ident[:],
                                rhs=bias_t[:, a - i0:bnd - i0],
                                start=False, stop=True,
                            )
                            nc.scalar.activation(
                                S[:, a:bnd], s_ps[:, a - h0c:bnd - h0c],
                                Act.Copy)
                        else:
                            nc.scalar.activation(
                                S[:, a:bnd], s_ps[:, a - h0c:bnd - h0c],
                                Act.Identity,
                                bias=clamps[qi][:, bc:bc + 1])
                S_l[qi] = S

            def stage_select(qi):
                S = S_l[qi]
                cands = small.tile([128, 256], F32, tag="cands")
                for ci in range(32):
                    nc.vector.max(out=cands[:, ci * 8:(ci + 1) * 8],
                                  in_=S[:, ci * 32:(ci + 1) * 32])
                mv = small.tile([128, 8], F32, tag="mv")
                for r in range(8):
                    nc.vector.max(out=mv[:], in_=cands[:])
                    if r < 7:
                        nc.vector.match_replace(out=cands[:], in_to_replace=mv[:],
                                                in_values=cands[:], imm_value=NEG)
                tp = mv[:, 7:8]
                negt = small.tile([128, 1], F32, tag="negt")
                nc.gpsimd.tensor_scalar(negt[:], tp, -1.0, None, op0=Alu.mult)
                mneg = mpool.tile([128, N], BF16, tag="mneg")
                nc.gpsimd.tensor_scalar(mneg[:], S[:], tp, NEG,
                                        op0=Alu.is_lt, op1=Alu.mult)
                nc.gpsimd.tensor_tensor(out=S[:], in0=S[:], in1=mneg[:], op=Alu.add)
                negt_l[qi] = negt

            def stage_exp(qi):
                i0 = qi * 128
                S, negt = S_l[qi], negt_l[qi]
                E = epool.tile([128, N], BF16, tag="E")
                den = small.tile([128, 1], F32, tag="den")
                nc.scalar.activation(E[:], S[:], Act.Exp, bias=negt[:],
                                     scale=1.0, accum_out=den[:])
                rden = small.tile([128, 1], F32, tag="rden")
                nc.vector.reciprocal(rden[:], den[:])
                # avrep psum: rows [0:64] = V^T A^T, rows [64:128] = rden
                # broadcast along the free dim (rep[p, i] = rden[i])
                avrep = ps_av.tile([128, 128], F32, tag="avrep")
                nc.tensor.matmul(
                    avrep[DH:2 * DH, :],
                    lhsT=AP(tensor=rden.tensor, offset=rden.offset,
                            ap=[list(rden.ap[0]), [0, DH]]),
                    rhs=identf[:],
                    start=True, stop=True,
                )
                rep_sb = repp.tile([DH, 128], F32, tag="rep")
                nc.scalar.activation(rep_sb[:], avrep[DH:2 * DH, :], Act.Copy)
                G = gpool.tile([128, N], BF16, tag="G")
                nc.sync.dma_start(out=G[:], in_=gat[b, h, i0:i0 + 128, :])
                A = epool.tile([128, N], BF16, tag="A")
                nc.gpsimd.tensor_tensor(out=A[:], in0=E[:], in1=G[:], op=Alu.mult)
                avrep_l[qi], A_l[qi] = (avrep, rep_sb), A

            def stage_av(qi):
                i0 = qi * 128
                A, (avrep, rep_sb) = A_l[qi], avrep_l[qi]
                t_ps = ps_t.tile([128, 8, 128], BF16, tag="tr")
                for jc in range(8):
                    nc.tensor.transpose(t_ps[:, jc, :], A[:, jc * 128:(jc + 1) * 128], ident[:])
                At = atp.tile([128, 8, 128], BF16, tag="At")
                nc.scalar.activation(At[:], t_ps[:], Act.Copy)
                for jc in range(8):
                    nc.tensor.matmul(
                        avrep[0:DH, :],
                        lhsT=V[:, jc, hs:hs + DH],
                        rhs=At[:, jc, :],
                        start=(jc == 0), stop=(jc == 7),
                    )
                nc.vector.tensor_tensor(out=OT[hs:hs + DH, i0:i0 + 128],
                                        in0=avrep[0:DH, :],
                                        in1=rep_sb[:], op=Alu.mult)

            nslots = NQT + 2
            pending = list(extra_units)
            for t in range(nslots):
                take = -(-len(pending) // (nslots - t)) if pending else 0
                for _ in range(take):
                    pending.pop(0)()
                if t < NQT:
                    stage_scores(t)
                    stage_select(t)
                if 1 <= t <= NQT:
                    stage_exp(t - 1)
                if t >= 2:
                    stage_av(t - 2)

        # --- batch pipeline: proj(b+1) / P-phases / out_proj(b-1) are
        # interleaved into the tile loops as units ---
        xt_cur = emit_xt(0)
        qT, kT, V, u0 = proj_units(xt_cur)
        for u in u0:
            u()
        clamps_cur, pu = p_units(qT, 0, pexts[0])
        for u in pu:
            u()

        nxt = {}
        for b in range(B):
            OT = otp.tile([128, N], BF16, tag="OT")
            # h = 0
            extra = []
            if b + 1 < B:
                nxt["xt"] = emit_xt(b + 1)
            if b > 0:
                extra += nxt.pop("outproj")
            clamps_h1, pu = p_units(qT, DH, pexts[(2 * b + 1) % 4])
            extra += pu
            tile_loop(b, 0, qT, kT, V, OT, clamps_cur, pexts[(2 * b) % 4], extra)
            # h = 1
            extra = []
            if b + 1 < B:
                nqT, nkT, nV, units = proj_units(nxt.pop("xt"))
                extra += units
                clamps_cur, pu = p_units(nqT, 0, pexts[(2 * b + 2) % 4])
                extra += pu
            tile_loop(b, 1, qT, kT, V, OT, clamps_h1, pexts[(2 * b + 1) % 4], extra)
            if b + 1 < B:
                qT, kT, V = nqT, nkT, nV
            nxt["outproj"] = outproj_units(b, OT)
        for u in nxt.pop("outproj"):
            u()


    split_excess_waits(nc)
    return nc


_CACHED = {}


def _get_program():
    if "nc" not in _CACHED:
        _apply_tile_patch()
        _CACHED["nc"] = build_program()
    return _CACHED["nc"]


def _make_in_maps(x, gating_mask, Wq, bq, Wkv, bkv, Wo, rel_emb):
    xT = np.ascontiguousarray(x.transpose(0, 2, 1))            # [B, DIM, N]
    # NOTE: q is pre-scaled by SCALE via Wq, which already covers the
    # rel-pos bias term (bias = q_scaled . rel_emb) — do NOT scale reT too.
    reTs = np.ascontiguousarray(rel_emb.T)                     # [DH, MAX_POS]
    ident = np.eye(128, dtype=np.float32)

    import ml_dtypes

    def bf16(a):
        return a.astype(ml_dtypes.bfloat16)

    in_maps = []
    for c in range(NCORES):
        h0 = c * HPC
        cols = slice(h0 * DH, (h0 + HPC) * DH)
        wq_c = np.ascontiguousarray(Wq[:, cols] * SCALE)
        wk_c = np.ascontiguousarray(Wkv[:, h0 * DH:(h0 + HPC) * DH])
        wv_c = np.ascontiguousarray(Wkv[:, INNER + h0 * DH:INNER + (h0 + HPC) * DH])
        bq_c = bq[cols] * SCALE
        bk_c = bkv[h0 * DH:(h0 + HPC) * DH]
        bv_c = bkv[INNER + h0 * DH:INNER + (h0 + HPC) * DH]
        bqk_c = np.ascontiguousarray(np.stack([bq_c, bk_c], axis=1))
        wo_c = np.ascontiguousarray(Wo[cols, :])
        gat_c = np.ascontiguousarray(gating_mask[:, h0:h0 + HPC])
        in_maps.append({
            "xT": xT,
            "wq": wq_c, "wk": wk_c, "wv": wv_c,
            "bqk": bqk_c.astype(np.float32),
            "bvb": bv_c.reshape(1, -1).astype(np.float32),
            "wo": bf16(wo_c),
            "reT": reTs,
            "gat": bf16(gat_c),
            "ident": bf16(ident),
            "identf": ident,
        })
    return in_maps


def time_kernel(inputs, repeats=5, rounds=1, gap_s=0.0):
    """Device-side timing: pre-stage sharded inputs on the 8 cores and re-run
    the jitted sharded executable; report min wall-clock in ns.

    Dispatch overhead through the PJRT proxy fluctuates in multi-second
    phases; sampling `rounds` batches of `repeats` with `gap_s` sleeps
    between batches makes the min a much more stable estimate of the
    per-execution floor."""
    import time as _time
    import jax
    import concourse.mybir as mb
    from concourse import bass2jax
    from jax.sharding import Mesh, PartitionSpec
    from jax.experimental.shard_map import shard_map

    x = np.asarray(inputs["x"], np.float32)
    in_maps = _make_in_maps(
        x, np.asarray(inputs["gating_mask"], np.float32),
        np.asarray(inputs["Wq"], np.float32), np.asarray(inputs["bq"], np.float32),
        np.asarray(inputs["Wkv"], np.float32), np.asarray(inputs["bkv"], np.float32),
        np.asarray(inputs["Wo"], np.float32), np.asarray(inputs["rel_emb"], np.float32))
    nc = _get_program()
    bass2jax.install_neuronx_cc_hook()
    n_cores = NCORES
    partition_name = nc.partition_id_tensor.name if nc.partition_id_tensor else None
    in_names, out_names, out_avals, zero_outs = [], [], [], []
    for alloc in nc.m.functions[0].allocations:
        if not isinstance(alloc, mb.MemoryLocationSet):
            continue
        name = alloc.memorylocations[0].name
        if alloc.kind == "ExternalInput":
            if name != partition_name:
                in_names.append(name)
        elif alloc.kind == "ExternalOutput":
            shape = tuple(alloc.tensor_shape)
            dtype = mb.dt.np(alloc.dtype)
            out_names.append(name)
            out_avals.append(jax.core.ShapedArray(shape, dtype))
            zero_outs.append(np.zeros(shape, dtype))
    n_params = len(in_names)
    n_outs = len(out_avals)
    all_in_names = list(in_names) + out_names
    if partition_name is not None:
        all_in_names.append(partition_name)

    def _body(*args):
        operands = list(args)
        if partition_name is not None:
            operands.append(bass2jax.partition_id_tensor())
        return tuple(bass2jax._bass_exec_p.bind(
            *operands,
            out_avals=tuple(out_avals), in_names=tuple(all_in_names),
            out_names=tuple(out_names), lowering_input_output_aliases=(),
            sim_require_finite=True, sim_require_nnan=True, nc=nc,
        ))

    devices = jax.devices()[:n_cores]
    mesh = Mesh(np.asarray(devices), ("core",))
    in_specs = (PartitionSpec("core"),) * (n_params + n_outs)
    out_specs = (PartitionSpec("core"),) * n_outs
    sharded = jax.jit(
        shard_map(_body, mesh=mesh, in_specs=in_specs, out_specs=out_specs,
                  check_rep=False),
        donate_argnums=tuple(range(n_params, n_params + n_outs)),
        keep_unused=True)
    concat_in = [
        np.concatenate([np.asarray(in_maps[c][nm]) for c in range(n_cores)], axis=0)
        for nm in in_names
    ]
    sharding = jax.sharding.NamedSharding(mesh, PartitionSpec("core"))
    dev_in = [jax.device_put(a, sharding) for a in concat_in]
    times = []
    for r in range(rounds):
        if r and gap_s:
            _time.sleep(gap_s)
        for _ in range(repeats):
            zeros = [jax.device_put(
                np.zeros((n_cores * z.shape[0], *z.shape[1:]), z.dtype), sharding)
                for z in zero_outs]
            for z in zeros:
                z.block_until_ready()
            t0 = _time.perf_counter()
            outs = sharded(*dev_in, *zeros)
            for o in outs:
                o.block_until_ready()
            times.append(_time.perf_counter() - t0)
    return min(times) * 1e9


def kernel(x, mask, gating_mask, Wq, bq, Wkv, bkv, Wo, bo, rel_emb, _trace=False):
    x = np.asarray(x, np.float32)
    gating_mask = np.asarray(gating_mask, np.float32)
    Wq = np.asarray(Wq, np.float32)
    bq = np.asarray(bq, np.float32)
    Wkv = np.asarray(Wkv, np.float32)
    bkv = np.asarray(bkv, np.float32)
    Wo = np.asarray(Wo, np.float32)
    bo = np.asarray(bo, np.float32)
    rel_emb = np.asarray(rel_emb, np.float32)
    assert np.asarray(mask).all(), "kernel assumes all-ones padding mask"

    nc = _get_program()
    in_maps = _make_in_maps(x, gating_mask, Wq, bq, Wkv, bkv, Wo, rel_emb)
    res = run_bass_kernel_spmd(nc, in_maps, list(range(NCORES)))
    outs = [np.asarray(r["out"], np.float32) for r in res.results]
    total = np.sum(outs, axis=0) + bo[None, None, :]
    return total.astype(np.float32)

